# revision 11
# baseline (speedup 1.0000x reference)
"""Trainium2 Bass kernel for nn_PointWiseGlobalFusion (v2, bf16 compute).

Reference computation (B=2, N=5, C=64, H=W=256, G=4 groups, nf=64):
  emb1 = grouped_conv3x3(x, w1, b1); emb2 = grouped_conv3x3(x, w2, b2)
  cor[n,m,g,h,w] = sum_c emb1[n,g,c,h,w] * emb2[m,g,c,h,w]
  att  = softmax_m(cor);  nl[n] = sum_m att[n,m] * x[m]
  out  = concat([x, nl, broadcast(max_n x)], channel axis)

Sharding: 8 cores = (b in {0,1}) x (4 H-quarters of 64 rows).

Per-core kernel design:
  xb  bf16 [128, 5, 66, 258]: rows 0-63 = padded x channels (dy=0 view),
      rows 64-127 = same shifted one image row down (dy=1 view).  Both
      grouped convs = 6 matmuls/frame: 3 dx-shifts over the dy{0,1} pair
      (K=128) + 3 dx-shifts for dy=2 (upper-half weights zero).
  E / Eswap bf16: (emb1|emb2) and (emb2|emb1); tensor_mul of E[:,i] with
      Eswap[:,j] yields products for ordered pairs (i,j) AND (j,i) ->
      15 muls cover all 25 pairs.  Per-pair partition-sums over c go
      through mask-lhsT matmuls, 4-way column-tiled on the PE.
  softmax over frames via mask matmuls + ACT exp + fast reciprocal.
  weighted sum: att broadcast over c by mask matmuls (pair-stacked),
      DVE multiply with duplicated-channel x, accumulation over frames
      via identity matmuls into PSUM.
"""

import os
import sys

import numpy as np

if "/opt/trn_rl_repo" not in sys.path:
    sys.path.insert(0, "/opt/trn_rl_repo")

import ml_dtypes

BF16 = ml_dtypes.bfloat16

B, N, C, H, W = 2, 5, 64, 256, 256
G, CG = 4, 16
HQ = H // 4
NPAIR = N * N

RS = 4  # rows per strip
NSTRIP = HQ // RS
RC = 2  # rows per psum chunk
CPX = RC * W  # 512
SPX = RS * W  # 1024

# correlation product stacks: unordered pairs, ordered so each stack's
# cor-psum rows stay inside one 32-partition column group (for col-tiling)
ND = [(i, j) for i in range(N) for j in range(i + 1, N)]  # 10 stacks, 8 rows
DG = [(i, i) for i in range(N)]  # 5 stacks, 4 rows
STACKS = ND[0:4] + ND[4:8] + ND[8:10] + DG[0:4] + DG[4:5]
GRP_OF = [0] * 4 + [1] * 4 + [2] * 6 + [3] * 1
ROWPOS = {}
STACK_OFF = []
_off = 0
for _u, (_i, _j) in enumerate(STACKS):
    STACK_OFF.append(_off)
    ROWPOS[(_i, _j)] = _off
    if _i != _j:
        ROWPOS[(_j, _i)] = _off + 4
        _off += 8
    else:
        _off += 4
assert _off == 100

NQ = [(0, 1), (2, 3), (4, None)]  # n-pair stacks for the weighted sum

_CACHE = {}


def _masks():
    # per-stack cor-reduce lhsT: product rows -> 32 cols of the stack's grp
    red2 = np.zeros((15, 128, 32), dtype=np.float32)
    for u, (i, j) in enumerate(STACKS):
        base = GRP_OF[u] * 32
        for c in range(C):
            red2[u, c, ROWPOS[(i, j)] + c // CG - base] = 1.0
            if i != j:
                red2[u, C + c, ROWPOS[(j, i)] + c // CG - base] = 1.0
    summ = np.zeros((100, 4 * N), dtype=np.float32)
    rbm = np.zeros((4 * N, 100), dtype=np.float32)
    for n in range(N):
        for m in range(N):
            for g in range(G):
                summ[ROWPOS[(n, m)] + g, 4 * n + g] = 1.0
                rbm[4 * n + g, ROWPOS[(n, m)] + g] = 1.0
    attb = np.zeros((3 * N, 100, 128), dtype=np.float32)
    for q, (a, b) in enumerate(NQ):
        for m in range(N):
            t = q * N + m
            for g in range(G):
                attb[t, ROWPOS[(a, m)] + g, g * CG : (g + 1) * CG] = 1.0
                if b is not None:
                    attb[t, ROWPOS[(b, m)] + g, C + g * CG : C + (g + 1) * CG] = 1.0
    ident = np.eye(128, dtype=np.float32)
    return red2, summ, rbm, attb, ident


def _conv_weights(w1, b1, w2, b2):
    """wt[6, 128, 128] lhsT: slots 0-2 dx with dy{0,1}; slots 3-5 dx, dy2."""
    wt = np.zeros((6, 128, 2 * C), dtype=np.float32)
    for dx in range(3):
        for o in range(C):
            g = o // CG
            r = slice(g * CG, (g + 1) * CG)
            r2 = slice(C + g * CG, C + (g + 1) * CG)
            wt[dx, r, o] = w1[o, :, 0, dx]
            wt[dx, r, C + o] = w2[o, :, 0, dx]
            wt[dx, r2, o] = w1[o, :, 1, dx]
            wt[dx, r2, C + o] = w2[o, :, 1, dx]
            wt[3 + dx, r, o] = w1[o, :, 2, dx]
            wt[3 + dx, r, C + o] = w2[o, :, 2, dx]
    b12 = np.concatenate([b1, b2]).astype(np.float32).reshape(128, 1)
    return wt, b12


def _build_bass():
    import concourse.bacc as bacc
    import concourse.mybir as mybir
    from concourse import tile

    F32 = mybir.dt.float32
    BF = mybir.dt.bfloat16
    IDN = mybir.ActivationFunctionType.Identity
    EXP = mybir.ActivationFunctionType.Exp
    nc = bacc.Bacc("TRN2", target_bir_lowering=False)

    xb_in = nc.dram_tensor("xb", [128, N, HQ + 2, W + 2], BF, kind="ExternalInput")
    xw_in = nc.dram_tensor("xw", [128, N, HQ, W], BF, kind="ExternalInput")
    xp_in = nc.dram_tensor("xpass", [N, C, HQ, W], F32, kind="ExternalInput")
    wt_in = nc.dram_tensor("wt", [6, 128, 128], BF, kind="ExternalInput")
    b12_in = nc.dram_tensor("b12", [128, 1], F32, kind="ExternalInput")
    red_in = nc.dram_tensor("red2", [15, 128, 32], BF, kind="ExternalInput")
    summ_in = nc.dram_tensor("summ", [100, 4 * N], BF, kind="ExternalInput")
    rbm_in = nc.dram_tensor("rbm", [4 * N, 100], BF, kind="ExternalInput")
    attb_in = nc.dram_tensor("attb", [15, 100, 128], BF, kind="ExternalInput")
    id_in = nc.dram_tensor("ident", [128, 128], BF, kind="ExternalInput")
    out = nc.dram_tensor("out", [N, 3 * C, HQ, W], F32, kind="ExternalOutput")

    with tile.TileContext(nc) as tc:
        with (
            tc.tile_pool(name="const", bufs=1) as cp,
            tc.tile_pool(name="xsb", bufs=2) as xp,
            tc.tile_pool(name="emb", bufs=2) as ep,
            tc.tile_pool(name="prod", bufs=1) as prp,
            tc.tile_pool(name="small", bufs=3) as sp,
            tc.tile_pool(name="psE", bufs=2, space="PSUM") as psE,
            tc.tile_pool(name="psC", bufs=2, space="PSUM") as psC,
            tc.tile_pool(name="psA", bufs=1, space="PSUM") as psA,
            tc.tile_pool(name="psR", bufs=1, space="PSUM") as psR,
            tc.tile_pool(name="psN", bufs=2, space="PSUM") as psN,
        ):
            wt_t = cp.tile([128, 6, 128], BF)
            nc.sync.dma_start(wt_t[:], wt_in[:].transpose([1, 0, 2]))
            b12_t = cp.tile([128, 1], F32)
            nc.sync.dma_start(b12_t[:], b12_in[:])
            red_t = cp.tile([128, 15, 32], BF)
            nc.sync.dma_start(red_t[:], red_in[:].transpose([1, 0, 2]))
            summ_t = cp.tile([100, 4 * N], BF)
            nc.sync.dma_start(summ_t[:], summ_in[:])
            rbm_t = cp.tile([4 * N, 100], BF)
            nc.sync.dma_start(rbm_t[:], rbm_in[:])
            attb_t = cp.tile([100, 15, 128], BF)
            nc.sync.dma_start(attb_t[:], attb_in[:].transpose([1, 0, 2]))
            id_t = cp.tile([128, 128], BF)
            nc.sync.dma_start(id_t[:], id_in[:])

            # exact passthrough: DRAM -> DRAM
            for f in range(N):
                nc.sync.dma_start(out[f, 0:C, :, :], xp_in[f, :, :, :])

            grp_stacks = [[u for u in range(15) if GRP_OF[u] == g] for g in range(4)]
            nrounds = max(len(gs) for gs in grp_stacks)

            for s in range(NSTRIP):
                r0 = s * RS
                xbs = xp.tile([128, N, RS + 2, W + 2], BF, tag="xbs")
                nc.sync.dma_start(xbs[:], xb_in[:, :, r0 : r0 + RS + 2, :])
                xws = xp.tile([128, N, RS, W], BF, tag="xws")
                nc.sync.dma_start(xws[:], xw_in[:, :, r0 : r0 + RS, :])

                # frame-max pool (bf16 tree, final step casts to f32)
                pa = sp.tile([C, 2, SPX], BF, tag="poolA")
                pf = sp.tile([C, SPX], F32, tag="poolF")
                nc.vector.tensor_max(pa[:], xws[0:C, 0:2, :, :], xws[0:C, 2:4, :, :])
                nc.vector.tensor_max(pa[:, 0, :], pa[:, 0, :], pa[:, 1, :])
                nc.vector.tensor_max(pf[:], pa[:, 0, :], xws[0:C, 4, :, :])
                for f in range(N):
                    nc.sync.dma_start(out[f, 2 * C : 3 * C, r0 : r0 + RS, :], pf[:])

                # ---- both grouped convs, all 5 frames, 4 rows ----
                E = ep.tile([128, N, SPX], BF, tag="E")
                Es = ep.tile([128, N, SPX], BF, tag="Es")
                for f in range(N):
                    for rc2 in range(RS // RC):
                        pe = psE.tile([128, CPX], F32)
                        for j in range(6):
                            dy0 = 0 if j < 3 else 2
                            dx = j % 3
                            rhs = xbs[
                                :, f, rc2 * RC + dy0 : rc2 * RC + dy0 + RC, dx : dx + W
                            ]
                            nc.tensor.matmul(
                                pe[:], wt_t[:, j, :], rhs, start=(j == 0), stop=(j == 5)
                            )
                        sl = slice(rc2 * CPX, (rc2 + 1) * CPX)
                        nc.scalar.activation(E[:, f, sl], pe[:], IDN, bias=b12_t[:, 0:1])
                # swapped copy (emb2|emb1) for pair-stacked products
                nc.vector.tensor_copy(Es[0:C, :, :], E[C : 2 * C, :, :])
                nc.vector.tensor_copy(Es[C : 2 * C, :, :], E[0:C, :, :])

                # ---- correlation products (both chunks at once) ----
                prods = prp.tile([128, 15, SPX], BF, tag="prods")
                for u, (i, j) in enumerate(STACKS):
                    nc.vector.tensor_mul(prods[:, u, :], E[:, i, :], Es[:, j, :])

                for rc2 in range(RS // RC):
                    sl = slice(rc2 * CPX, (rc2 + 1) * CPX)
                    rows = slice(r0 + rc2 * RC, r0 + rc2 * RC + RC)

                    # col-tiled mask matmuls accumulate cor into one bank
                    pc = psC.tile([100, CPX], F32)
                    for r in range(nrounds):
                        for g in range(4):
                            if r >= len(grp_stacks[g]):
                                continue
                            u = grp_stacks[g][r]
                            hi = min(32 * g + 32, 100)
                            nc.tensor.matmul(
                                pc[32 * g : hi, :],
                                red_t[:, u, 0 : hi - 32 * g],
                                prods[:, u, sl],
                                start=(r == 0),
                                stop=(r == len(grp_stacks[g]) - 1),
                                tile_position=(0, 32 * g),
                                skip_group_check=True,
                            )

                    exps = sp.tile([100, CPX], BF, tag="exps")
                    nc.scalar.activation(exps[:], pc[:], EXP)
                    ps_s = psA.tile([128, CPX], F32, tag="attb")
                    nc.tensor.matmul(
                        ps_s[0 : 4 * N, :], summ_t[:], exps[:], start=True, stop=True
                    )
                    rec = sp.tile([4 * N, CPX], F32, tag="rec")
                    recs = sp.tile([4 * N, CPX], F32, tag="recs")
                    nc.vector.reciprocal_approx_accurate(rec[:], ps_s[0 : 4 * N, :], recs[:])
                    recb = sp.tile([4 * N, CPX], BF, tag="recb")
                    nc.vector.tensor_copy(recb[:], rec[:])
                    ps_rb = psR.tile([100, CPX], F32)
                    nc.tensor.matmul(ps_rb[:], rbm_t[:], recb[:], start=True, stop=True)
                    att = sp.tile([100, CPX], BF, tag="att")
                    nc.vector.tensor_mul(att[:], exps[:], ps_rb[:])

                    # ---- weighted sum over frames, n-pair stacked ----
                    for q, (a, b) in enumerate(NQ):
                        pnl = psN.tile([128, CPX], F32)
                        for m in range(N):
                            t = q * N + m
                            pab = psA.tile([128, CPX], F32, tag="attb")
                            nc.tensor.matmul(
                                pab[:], attb_t[:, t, :], att[:], start=True, stop=True
                            )
                            pw = sp.tile([128, CPX], BF, tag="pw")
                            xm = xws[:, m, rc2 * RC : rc2 * RC + RC, :]
                            nc.vector.tensor_mul(pw[:], pab[:], xm)
                            nc.tensor.matmul(
                                pnl[:], id_t[:], pw[:], start=(m == 0), stop=(m == N - 1)
                            )
                        nlf = sp.tile([128, CPX], F32, tag="nlf")
                        nc.scalar.copy(nlf[:], pnl[:])
                        nc.sync.dma_start(out[a, C : 2 * C, rows, :], nlf[0:C, :])
                        if b is not None:
                            nc.sync.dma_start(out[b, C : 2 * C, rows, :], nlf[C : 2 * C, :])

    nc.compile()
    return nc


def _get_nc():
    if "nc" not in _CACHE:
        _CACHE["nc"] = _build_bass()
    return _CACHE["nc"]


def _shard_x(x):
    xpad = np.pad(x, ((0, 0), (0, 0), (0, 0), (1, 1), (1, 1)))
    shards = []
    for core in range(8):
        b, q = divmod(core, 4)
        sl = xpad[b, :, :, q * HQ : q * HQ + HQ + 2, :]
        slt = sl.transpose(1, 0, 2, 3)  # [64, 5, 66, 258]
        xb = np.zeros((128, N, HQ + 2, W + 2), dtype=BF16)
        xb[0:C] = slt.astype(BF16)
        xb[C:, :, 0 : HQ + 1, :] = slt[:, :, 1:, :].astype(BF16)
        xc = x[b, :, :, q * HQ : (q + 1) * HQ, :]
        xct = xc.transpose(1, 0, 2, 3).astype(BF16)
        xw = np.concatenate([xct, xct], axis=0)
        shards.append(
            {
                "xb": np.ascontiguousarray(xb),
                "xw": np.ascontiguousarray(xw),
                "xpass": np.ascontiguousarray(xc.astype(np.float32)),
            }
        )
    return shards


def _ensure_ntff_hook():
    import types

    try:
        from antenv.axon_hooks import get_axon_ntff_profile_hook  # noqa: F401

        return
    except ImportError:
        pass
    import antenv

    mod = types.ModuleType("antenv.axon_hooks")
    _state = {"hook": None}
    mod.set_axon_ntff_profile_hook = lambda h: _state.__setitem__("hook", h)
    mod.get_axon_ntff_profile_hook = lambda: _state["hook"]
    sys.modules["antenv.axon_hooks"] = mod
    antenv.axon_hooks = mod
    try:
        from trn_agent_boot.trn_boot import _ntff_profile_via_ctypes

        mod.set_axon_ntff_profile_hook(
            _ntff_profile_via_ctypes("/opt/axon/libaxon_pjrt.so")
        )
    except Exception as e:
        print(f"ntff hook setup failed: {e}", file=sys.stderr)


def kernel(x, w1, b1, w2, b2):
    from concourse.bass_utils import run_bass_kernel_spmd

    x = np.asarray(x, dtype=np.float32)
    nc = _get_nc()
    wt, b12 = _conv_weights(
        np.asarray(w1, np.float32), np.asarray(b1, np.float32),
        np.asarray(w2, np.float32), np.asarray(b2, np.float32),
    )
    red2, summ, rbm, attb, ident = _masks()
    consts = {
        "wt": wt.astype(BF16), "b12": b12,
        "red2": red2.astype(BF16), "summ": summ.astype(BF16),
        "rbm": rbm.astype(BF16), "attb": attb.astype(BF16),
        "ident": ident.astype(BF16),
    }
    shards = _shard_x(x)
    in_maps = [dict(shards[i], **consts) for i in range(8)]
    trace = bool(int(os.environ.get("KERNEL_TRACE", "0")))
    if trace:
        _ensure_ntff_hook()
    res = run_bass_kernel_spmd(nc, in_maps, list(range(8)), trace=trace)
    if trace:
        print(f"HW exec time: {res.exec_time_ns} ns (mean {res.mean_exec_time_ns})")
        _CACHE["last_results"] = res

    full = np.empty((B, N, 3 * C, H, W), dtype=np.float32)
    for core in range(8):
        b, q = divmod(core, 4)
        full[b, :, :, q * HQ : (q + 1) * HQ, :] = res.results[core]["out"]
    return full
